# revision 4
# baseline (speedup 1.0000x reference)
"""Segment-mean (weighted segment sum, pow=-1) Trainium2 kernel.

Problem: feats [16, 8192, 512] f32, seg_ids [16, 8192] sorted ints in [0, 2048)
-> out [16, 2048, 512] f32 where out[b, g] = mean of feats[b, s] over tokens s
with seg_ids[b, s] == g (0 for empty groups).

Strategy: data-parallel over batch (2 batches per core, 8 cores; the batch ->
(core, slot) assignment is chosen by a 2-opt pass that minimizes the SPMD
union schedule size). Per batch, groups are processed in 16 aligned windows
of 128. For each 128-token tile that intersects a window, build a one-hot
matrix W[t, g] = (sidw[t] == g) on the vector engine (sidw = seg_id -
window_base, precomputed per pair on the host) and accumulate W.T @
feats_tile into PSUM on the tensor engine.

The kernel is HBM-bandwidth bound, so all bulk I/O is fp16: feats are
downcast on the host into a token-major [tok, (bs, tile, h)] layout (fully
contiguous per partition), and the output is written fp16 in partition-major
[bs, tok, window, 512] layout and transposed/upcast on the host. Inverse
group counts are exact, computed on the host from seg_ids and applied as the
per-partition scale on the PSUM -> SBUF copy (no on-device count matmuls).

Every feats chunk has a dedicated SBUF buffer, so all load triggers are
emitted up-front with no buffer-recycling waits, alternating between the two
HWDGE rings (SP and ACT). The first chunks are small so all 16 SDMA engines
engage early; the last chunk is small so the final window's compute starts
as soon as possible after the last byte lands.
"""

import os
import sys

sys.path.insert(0, "/opt/trn_rl_repo")

import numpy as np

import concourse.bacc as bacc
import concourse.bass as bass
import concourse.mybir as mybir
from concourse import bass_utils, tile
from concourse.alu_op_type import AluOpType

B, S, H, G = 16, 8192, 512, 2048
N_CORES = 8
BPC = B // N_CORES        # batches per core
TOK = 128                 # tokens per tile
NT = S // TOK             # 64 token tiles per batch
WIN = 128                 # groups per window
NW = G // WIN             # 16 windows per batch

# chunk plan: (tile_start, n_tiles) per batch slot. Small head chunks ramp
# the SDMA engines up quickly; small tail chunks shrink the final-window
# dependency.
CHUNKS = {
    0: [(0, 2), (2, 2), (4, 4), (8, 8), (16, 8), (24, 8), (32, 8), (40, 8),
        (48, 8), (56, 8)],
    1: [(0, 8), (8, 8), (16, 8), (24, 8), (32, 8), (40, 8), (48, 8), (56, 4),
        (60, 2), (62, 2)],
}

fp32 = mybir.dt.float32
fp16 = mybir.dt.float16
i32 = mybir.dt.int32

_NC_CACHE = {}
LAST_RESULTS = None


def _build_program(union_tiles, npairs):
    """union_tiles[bs][j] = tuple of token-tile indices feeding window j.

    Pair q (in emission order) compares token tile i against window j via
    sidw[:, q] = seg_id - 128*j, precomputed on the host.
    """
    nc = bacc.Bacc("TRN2", target_bir_lowering=False, debug=False,
                   num_devices=N_CORES)
    feats_d = nc.dram_tensor("feats", [TOK, BPC * NT * H], fp16,
                             kind="ExternalInput")
    sidw_d = nc.dram_tensor("sidw", [TOK, npairs], fp32,
                            kind="ExternalInput")
    inv_d = nc.dram_tensor("inv", [TOK, BPC * NW], fp32,
                           kind="ExternalInput")
    out_d = nc.dram_tensor("out", [BPC, TOK, NW * H], fp16,
                           kind="ExternalOutput")

    with tile.TileContext(nc) as tc:
        with (
            tc.tile_pool(name="const", bufs=1) as cpool,
            tc.tile_pool(name="feats", bufs=1) as fpool,
            tc.tile_pool(name="wpool", bufs=16) as wpool,
            tc.tile_pool(name="ostage", bufs=2) as opool,
            tc.tile_pool(name="pso", bufs=6, space=bass.MemorySpace.PSUM) as pso,
        ):
            # small inputs lead the ACT ring; feats chunks alternate rings
            sidw_sb = cpool.tile([TOK, npairs], fp32)
            nc.scalar.dma_start(sidw_sb[:], sidw_d[:])
            inv_sb = cpool.tile([TOK, BPC * NW], fp32)
            nc.scalar.dma_start(inv_sb[:], inv_d[:])

            iota_i = cpool.tile([TOK, WIN], i32)
            nc.gpsimd.iota(iota_i[:], pattern=[[1, WIN]], base=0,
                           channel_multiplier=0)
            iota_h = cpool.tile([TOK, WIN], fp16)
            nc.vector.tensor_copy(iota_h[:], iota_i[:])

            # all feats loads up-front: dedicated buffers, no recycling waits
            tilebuf = {}   # tile index -> (sbuf tile, col offset)
            ring = 0
            for bs in range(BPC):
                for (i0, nt) in CHUNKS[bs]:
                    t = fpool.tile([TOK, nt * H], fp16,
                                   name=f"fc_{bs}_{i0}", tag=f"fc_{bs}_{i0}")
                    src = feats_d[:, (bs * NT + i0) * H:
                                  (bs * NT + i0 + nt) * H]
                    eng = nc.sync if ring == 0 else nc.scalar
                    eng.dma_start(t[:], src)
                    ring ^= 1
                    for k in range(nt):
                        tilebuf[bs * NT + i0 + k] = (t, k * H)

            q = 0  # running pair index
            for bs in range(BPC):
                ostage = opool.tile([TOK, NW * H], fp16)

                def store_after(j, bs=bs, ostage=ostage):
                    # emit the output store that completes with window j
                    if bs == BPC - 1 and j >= NW - 4:
                        # stream the final windows out individually so the
                        # last store is small and starts early
                        nc.scalar.dma_start(
                            out_d[bs, :, j * H:(j + 1) * H],
                            ostage[:, j * H:(j + 1) * H])
                    elif j % 4 == 3:
                        j0 = j - 3
                        nc.scalar.dma_start(
                            out_d[bs, :, j0 * H:(j + 1) * H],
                            ostage[:, j0 * H:(j + 1) * H])

                for j in range(NW):
                    tiles = union_tiles[bs][j]
                    if not tiles:
                        nc.gpsimd.memset(ostage[:, j * H:(j + 1) * H], 0.0)
                        store_after(j)
                        continue
                    ps = pso.tile([TOK, H], fp32)
                    n = len(tiles)
                    for idx, i in enumerate(tiles):
                        ft, off = tilebuf[bs * NT + i]
                        w = wpool.tile([TOK, WIN], fp16)
                        nc.vector.tensor_scalar(
                            w[:], iota_h[:], sidw_sb[:, q:q + 1], None,
                            op0=AluOpType.is_equal)
                        q += 1
                        nc.tensor.matmul(ps[:], w[:], ft[:, off:off + H],
                                         start=idx == 0, stop=idx == n - 1)
                    inv_col = inv_sb[:, bs * NW + j:bs * NW + j + 1]
                    od = ostage[:, j * H:(j + 1) * H]
                    if bs == BPC - 1 and j >= NW - 2:
                        # split the final copies across scalar+vector to
                        # shorten the end-of-kernel critical path
                        hh = H // 2
                        nc.scalar.activation(
                            od[:, :hh], ps[:, :hh],
                            mybir.ActivationFunctionType.Copy, scale=inv_col)
                        nc.vector.tensor_scalar(
                            od[:, hh:], ps[:, hh:], inv_col, None,
                            op0=AluOpType.mult)
                    else:
                        nc.scalar.activation(
                            od, ps[:], mybir.ActivationFunctionType.Copy,
                            scale=inv_col)
                    store_after(j)
            assert q == npairs

    nc.compile()
    return nc


def _union_pairs(lo, hi, rows):
    """#pairs for a slot holding batch rows `rows` (tuple of batch indices)."""
    lo_u = lo[list(rows)].min(axis=0)
    hi_u = hi[list(rows)].max(axis=0)
    return int((hi_u - lo_u + 1).sum())


def _assign_batches(lo, hi):
    """Partition 16 batches into two slot-sets of 8 minimizing union pairs.

    Greedy 2-opt from the identity assignment; the (core, slot) grid is then
    filled slot-major. Returns perm where perm[c*BPC+bs] = original batch.
    """
    slot0 = list(range(0, B, 2))
    slot1 = list(range(1, B, 2))
    best = _union_pairs(lo, hi, slot0) + _union_pairs(lo, hi, slot1)
    improved = True
    while improved:
        improved = False
        for a in range(N_CORES):
            for b in range(N_CORES):
                s0 = slot0.copy()
                s1 = slot1.copy()
                s0[a], s1[b] = s1[b], s0[a]
                cost = _union_pairs(lo, hi, s0) + _union_pairs(lo, hi, s1)
                if cost < best:
                    best = cost
                    slot0, slot1 = s0, s1
                    improved = True
    perm = [0] * B
    for c in range(N_CORES):
        perm[c * BPC + 0] = slot0[c]
        perm[c * BPC + 1] = slot1[c]
    return perm


def _schedule(perm, lo, hi):
    """Union (over cores) of window -> token-tile lists, per batch slot."""
    union = []
    for bs in range(BPC):
        rows = [perm[c * BPC + bs] for c in range(N_CORES)]
        lo_u = lo[rows].min(axis=0)   # [NT]
        hi_u = hi[rows].max(axis=0)   # [NT]
        per_win = []
        for j in range(NW):
            per_win.append(tuple(
                i for i in range(NT) if lo_u[i] <= j <= hi_u[i]))
        union.append(tuple(per_win))
    return tuple(union)


def kernel(feats, seg_ids):
    global LAST_RESULTS
    feats = np.asarray(feats)
    sid_raw = np.asarray(seg_ids)

    sid3 = sid_raw.astype(np.int64).reshape(B, NT, TOK)
    lo = sid3[:, :, 0] // WIN      # [B, NT] first window each tile touches
    hi = sid3[:, :, -1] // WIN     # [B, NT] last window each tile touches
    perm = _assign_batches(lo, hi)
    union = _schedule(perm, lo, hi)
    pairs = [(bs, j, i) for bs in range(BPC) for j in range(NW)
             for i in union[bs][j]]
    npairs = len(pairs)

    key = (union, npairs)
    if key not in _NC_CACHE:
        _NC_CACHE[key] = _build_program(union, npairs)
    nc = _NC_CACHE[key]

    # host-side prep: fp16 feats in token-major [tok, (bs, tile, h)] layout
    f16 = feats.astype(np.float16)

    sid_i = sid_raw.astype(np.int32).reshape(B, NT, TOK)
    # exact group counts -> inverse weights
    counts = np.zeros((B, G), np.int64)
    for b in range(B):
        counts[b] = np.bincount(sid_raw[b].astype(np.int64), minlength=G)
    inv = np.where(counts > 0, 1.0 / np.maximum(counts, 1), 0.0).astype(
        np.float32).reshape(B, NW, WIN)

    in_maps = []
    for c in range(N_CORES):
        rows = [perm[c * BPC + bs] for bs in range(BPC)]
        # feats_t[p, ((bs, i), h)] = feats[rows[bs], i*TOK + p, h]
        fc = np.ascontiguousarray(
            f16[rows].reshape(BPC * NT, TOK, H).transpose(1, 0, 2).reshape(
                TOK, BPC * NT * H))
        # per-pair window-local seg ids: sidw[p, q] = sid[b, i, p] - 128*j
        sidw = np.empty((TOK, npairs), np.float32)
        for qi, (bs, j, i) in enumerate(pairs):
            sidw[:, qi] = sid_i[rows[bs], i] - WIN * j
        # inv_t[p, bs*NW + j] = inv[rows[bs], j, p]
        inv_t = np.ascontiguousarray(
            inv[rows].transpose(2, 0, 1).reshape(TOK, BPC * NW))
        in_maps.append({"feats": fc, "sidw": sidw, "inv": inv_t})

    trace = bool(os.environ.get("SEGRED_TRACE"))
    res = bass_utils.run_bass_kernel_spmd(
        nc, in_maps, core_ids=list(range(N_CORES)), trace=trace)
    LAST_RESULTS = res

    # out_d[bs, p, j*H + h] = out[perm[c*BPC+bs], 128*j + p, h]
    out = np.empty((B, G, H), np.float32)
    for c in range(N_CORES):
        o = res.results[c]["out"].reshape(BPC, TOK, NW, H)
        o = o.transpose(0, 2, 1, 3).reshape(BPC, G, H).astype(np.float32)
        for bs in range(BPC):
            out[perm[c * BPC + bs]] = o[bs]
    return out


# revision 5
# speedup vs baseline: 1.2622x; 1.2622x over previous
"""Segment-mean (weighted segment sum, pow=-1) Trainium2 kernel.

Problem: feats [16, 8192, 512] f32, seg_ids [16, 8192] sorted ints in [0, 2048)
-> out [16, 2048, 512] f32 where out[b, g] = mean of feats[b, s] over tokens s
with seg_ids[b, s] == g (0 for empty groups).

Strategy: data-parallel over batch (2 batches per core, 8 cores; the batch ->
(core, slot) assignment is chosen by a 2-opt pass that minimizes the SPMD
union schedule size). Per batch, groups are processed in 16 aligned windows
of 128. For each 128-token tile that intersects a window, build a one-hot
matrix W[t, g] = (sidw[t] == g) on the vector engine (sidw = seg_id -
window_base, precomputed per pair on the host) and accumulate W.T @
feats_tile into PSUM on the tensor engine.

The kernel is HBM-bandwidth bound, so all bulk I/O is fp16: feats are
downcast on the host into a token-major [tok, (bs, tile, h)] layout (fully
contiguous per partition), and the output is written fp16 in partition-major
[bs, tok, window, 512] layout and transposed/upcast on the host. Inverse
group counts are exact, computed on the host from seg_ids and applied as the
per-partition scale on the PSUM -> SBUF copy (no on-device count matmuls).

Every feats chunk has a dedicated SBUF buffer, so all load triggers are
emitted up-front with no buffer-recycling waits, alternating between the two
HWDGE rings (SP and ACT). The first chunks are small so all 16 SDMA engines
engage early; the last chunk is small so the final window's compute starts
as soon as possible after the last byte lands.
"""

import os
import sys

sys.path.insert(0, "/opt/trn_rl_repo")

import numpy as np

import concourse.bacc as bacc
import concourse.bass as bass
import concourse.mybir as mybir
from concourse import bass_utils, tile
from concourse.alu_op_type import AluOpType

B, S, H, G = 16, 8192, 512, 2048
N_CORES = 8
BPC = B // N_CORES        # batches per core
TOK = 128                 # tokens per tile
NT = S // TOK             # 64 token tiles per batch
WIN = 128                 # groups per window
NW = G // WIN             # 16 windows per batch

# chunk plan: (tile_start, n_tiles) per batch slot. Small head chunks ramp
# the SDMA engines up quickly; small tail chunks shrink the final-window
# dependency.
CHUNKS = {
    0: [(0, 2), (2, 2), (4, 4), (8, 8), (16, 8), (24, 8), (32, 8), (40, 8),
        (48, 8), (56, 8)],
    1: [(0, 8), (8, 8), (16, 8), (24, 8), (32, 8), (40, 8), (48, 8), (56, 4),
        (60, 2), (62, 2)],
}

fp32 = mybir.dt.float32
fp16 = mybir.dt.float16
i32 = mybir.dt.int32

_NC_CACHE = {}
LAST_RESULTS = None


def _build_program(union_tiles, npairs):
    """union_tiles[bs][j] = tuple of token-tile indices feeding window j.

    Pair q (in emission order) compares token tile i against window j via
    sidw[:, q] = seg_id - 128*j, precomputed on the host.
    """
    nc = bacc.Bacc("TRN2", target_bir_lowering=False, debug=False,
                   num_devices=N_CORES)
    feats_d = nc.dram_tensor("feats", [TOK, BPC * NT * H], fp16,
                             kind="ExternalInput")
    sidw_d = nc.dram_tensor("sidw", [TOK, npairs], fp32,
                            kind="ExternalInput")
    inv_d = nc.dram_tensor("inv", [TOK, BPC * NW], fp32,
                           kind="ExternalInput")
    out_d = nc.dram_tensor("out", [BPC, TOK, NW * H], fp16,
                           kind="ExternalOutput")

    with tile.TileContext(nc) as tc:
        with (
            tc.tile_pool(name="const", bufs=1) as cpool,
            tc.tile_pool(name="feats", bufs=1) as fpool,
            tc.tile_pool(name="wpool", bufs=16) as wpool,
            tc.tile_pool(name="ostage", bufs=2) as opool,
            tc.tile_pool(name="pso", bufs=6, space=bass.MemorySpace.PSUM) as pso,
        ):
            # small inputs lead the ACT ring; feats chunks alternate rings
            sidw_sb = cpool.tile([TOK, npairs], fp32)
            nc.scalar.dma_start(sidw_sb[:], sidw_d[:])
            inv_sb = cpool.tile([TOK, BPC * NW], fp32)
            nc.scalar.dma_start(inv_sb[:], inv_d[:])

            iota_i = cpool.tile([TOK, WIN], i32)
            nc.gpsimd.iota(iota_i[:], pattern=[[1, WIN]], base=0,
                           channel_multiplier=0)
            iota_h = cpool.tile([TOK, WIN], fp16)
            nc.vector.tensor_copy(iota_h[:], iota_i[:])

            # all feats loads up-front on the SP ring: dedicated buffers, no
            # recycling waits (the ACT ring is left for stores — HWDGE rings
            # execute FIFO, so stores must not queue behind late loads)
            tilebuf = {}   # tile index -> (sbuf tile, col offset)
            for bs in range(BPC):
                for (i0, nt) in CHUNKS[bs]:
                    t = fpool.tile([TOK, nt * H], fp16,
                                   name=f"fc_{bs}_{i0}", tag=f"fc_{bs}_{i0}")
                    src = feats_d[:, (bs * NT + i0) * H:
                                  (bs * NT + i0 + nt) * H]
                    nc.sync.dma_start(t[:], src)
                    for k in range(nt):
                        tilebuf[bs * NT + i0 + k] = (t, k * H)

            q = 0  # running pair index
            for bs in range(BPC):
                ostage = opool.tile([TOK, NW * H], fp16)

                def store_after(j, bs=bs, ostage=ostage):
                    # emit the output store that completes with window j
                    if bs == BPC - 1 and j >= NW - 4:
                        # stream the final windows out individually so the
                        # last store is small and starts early
                        nc.scalar.dma_start(
                            out_d[bs, :, j * H:(j + 1) * H],
                            ostage[:, j * H:(j + 1) * H])
                    elif j % 4 == 3:
                        j0 = j - 3
                        nc.scalar.dma_start(
                            out_d[bs, :, j0 * H:(j + 1) * H],
                            ostage[:, j0 * H:(j + 1) * H])

                for j in range(NW):
                    tiles = union_tiles[bs][j]
                    if not tiles:
                        nc.gpsimd.memset(ostage[:, j * H:(j + 1) * H], 0.0)
                        store_after(j)
                        continue
                    ps = pso.tile([TOK, H], fp32)
                    n = len(tiles)
                    for idx, i in enumerate(tiles):
                        ft, off = tilebuf[bs * NT + i]
                        w = wpool.tile([TOK, WIN], fp16)
                        nc.vector.tensor_scalar(
                            w[:], iota_h[:], sidw_sb[:, q:q + 1], None,
                            op0=AluOpType.is_equal)
                        q += 1
                        nc.tensor.matmul(ps[:], w[:], ft[:, off:off + H],
                                         start=idx == 0, stop=idx == n - 1)
                    inv_col = inv_sb[:, bs * NW + j:bs * NW + j + 1]
                    od = ostage[:, j * H:(j + 1) * H]
                    if bs == BPC - 1 and j >= NW - 2:
                        # split the final copies across scalar+vector to
                        # shorten the end-of-kernel critical path
                        hh = H // 2
                        nc.scalar.activation(
                            od[:, :hh], ps[:, :hh],
                            mybir.ActivationFunctionType.Copy, scale=inv_col)
                        nc.vector.tensor_scalar(
                            od[:, hh:], ps[:, hh:], inv_col, None,
                            op0=AluOpType.mult)
                    else:
                        nc.scalar.activation(
                            od, ps[:], mybir.ActivationFunctionType.Copy,
                            scale=inv_col)
                    store_after(j)
            assert q == npairs

    nc.compile()
    return nc


def _union_pairs(lo, hi, rows):
    """#pairs for a slot holding batch rows `rows` (tuple of batch indices)."""
    lo_u = lo[list(rows)].min(axis=0)
    hi_u = hi[list(rows)].max(axis=0)
    return int((hi_u - lo_u + 1).sum())


def _assign_batches(lo, hi):
    """Partition 16 batches into two slot-sets of 8 minimizing union pairs.

    Greedy 2-opt from the identity assignment; the (core, slot) grid is then
    filled slot-major. Returns perm where perm[c*BPC+bs] = original batch.
    """
    slot0 = list(range(0, B, 2))
    slot1 = list(range(1, B, 2))
    best = _union_pairs(lo, hi, slot0) + _union_pairs(lo, hi, slot1)
    improved = True
    while improved:
        improved = False
        for a in range(N_CORES):
            for b in range(N_CORES):
                s0 = slot0.copy()
                s1 = slot1.copy()
                s0[a], s1[b] = s1[b], s0[a]
                cost = _union_pairs(lo, hi, s0) + _union_pairs(lo, hi, s1)
                if cost < best:
                    best = cost
                    slot0, slot1 = s0, s1
                    improved = True
    perm = [0] * B
    for c in range(N_CORES):
        perm[c * BPC + 0] = slot0[c]
        perm[c * BPC + 1] = slot1[c]
    return perm


def _schedule(perm, lo, hi):
    """Union (over cores) of window -> token-tile lists, per batch slot."""
    union = []
    for bs in range(BPC):
        rows = [perm[c * BPC + bs] for c in range(N_CORES)]
        lo_u = lo[rows].min(axis=0)   # [NT]
        hi_u = hi[rows].max(axis=0)   # [NT]
        per_win = []
        for j in range(NW):
            per_win.append(tuple(
                i for i in range(NT) if lo_u[i] <= j <= hi_u[i]))
        union.append(tuple(per_win))
    return tuple(union)


def kernel(feats, seg_ids):
    global LAST_RESULTS
    feats = np.asarray(feats)
    sid_raw = np.asarray(seg_ids)

    sid3 = sid_raw.astype(np.int64).reshape(B, NT, TOK)
    lo = sid3[:, :, 0] // WIN      # [B, NT] first window each tile touches
    hi = sid3[:, :, -1] // WIN     # [B, NT] last window each tile touches
    perm = _assign_batches(lo, hi)
    union = _schedule(perm, lo, hi)
    pairs = [(bs, j, i) for bs in range(BPC) for j in range(NW)
             for i in union[bs][j]]
    npairs = len(pairs)

    key = (union, npairs)
    if key not in _NC_CACHE:
        _NC_CACHE[key] = _build_program(union, npairs)
    nc = _NC_CACHE[key]

    # host-side prep: fp16 feats in token-major [tok, (bs, tile, h)] layout
    f16 = feats.astype(np.float16)

    sid_i = sid_raw.astype(np.int32).reshape(B, NT, TOK)
    # exact group counts -> inverse weights
    counts = np.zeros((B, G), np.int64)
    for b in range(B):
        counts[b] = np.bincount(sid_raw[b].astype(np.int64), minlength=G)
    inv = np.where(counts > 0, 1.0 / np.maximum(counts, 1), 0.0).astype(
        np.float32).reshape(B, NW, WIN)

    in_maps = []
    for c in range(N_CORES):
        rows = [perm[c * BPC + bs] for bs in range(BPC)]
        # feats_t[p, ((bs, i), h)] = feats[rows[bs], i*TOK + p, h]
        fc = np.ascontiguousarray(
            f16[rows].reshape(BPC * NT, TOK, H).transpose(1, 0, 2).reshape(
                TOK, BPC * NT * H))
        # per-pair window-local seg ids: sidw[p, q] = sid[b, i, p] - 128*j
        sidw = np.empty((TOK, npairs), np.float32)
        for qi, (bs, j, i) in enumerate(pairs):
            sidw[:, qi] = sid_i[rows[bs], i] - WIN * j
        # inv_t[p, bs*NW + j] = inv[rows[bs], j, p]
        inv_t = np.ascontiguousarray(
            inv[rows].transpose(2, 0, 1).reshape(TOK, BPC * NW))
        in_maps.append({"feats": fc, "sidw": sidw, "inv": inv_t})

    trace = bool(os.environ.get("SEGRED_TRACE"))
    res = bass_utils.run_bass_kernel_spmd(
        nc, in_maps, core_ids=list(range(N_CORES)), trace=trace)
    LAST_RESULTS = res

    # out_d[bs, p, j*H + h] = out[perm[c*BPC+bs], 128*j + p, h]
    out = np.empty((B, G, H), np.float32)
    for c in range(N_CORES):
        o = res.results[c]["out"].reshape(BPC, TOK, NW, H)
        o = o.transpose(0, 2, 1, 3).reshape(BPC, G, H).astype(np.float32)
        for bs in range(BPC):
            out[perm[c * BPC + bs]] = o[bs]
    return out
